# revision 77
# baseline (speedup 1.0000x reference)
"""Trainium2 Bass kernel for an AttentionBlock (GroupNorm + 4-head self-attention + proj).

Sharding: 8 cores = 4 batches x 2 head-pairs. Core c handles batch c//2, heads
{2j, 2j+1} where j = c%2. Each core: groupnorm of x[b] (duplicated across the
pair of cores), QKV for its 128 channels, fp8 flash attention, partial
projection. Host sums the two partial projections per batch and adds the
residual + an effective bias b_proj + w_proj @ b_v (softmax weights sum to 1,
so the v-bias folds into the output bias exactly; the k-bias shifts every
score of a query equally and is dropped).

fp8 attention scheme (per head):
  Q,K scaled by sqrt(log2 e) and quantized to e4m3 in DoubleRow layout
  [32 dims, 2 slots, tokens], written directly by the gen matmuls
  (q-bias lands via a rank-1 ones matmul). Scores arrive as log2(e)*s in
  fp32 PSUM. p = exp(s/8 + B), B = (17.3123-56)/(8 log2 e), computed two
  ways, split across engines to balance load:
    ACT: Exp activation -> fp8 out (true exp, RNE quantize)
    DVE: byte = max(log2e*s, -17.3123) + 17.3123 -> uint8 out; the e4m3 bit
         pattern IS a piecewise-linear exp (Schraudolph). Convert is RNE.
  PV: DoubleRow over key-tile pairs (256 tokens per matmul), stationary V
  padded to M=128 with a ones column at 64 producing the softmax denominator.
All attention matmuls run at 0.5 cycles/column (fp8 DoubleRow). x is loaded
as fp8 (the residual is re-added from fp32 on the host). The score psum pool
(3 x [128,1024]) also hosts gen/proj tiles; O psum frees fast at qc end via
SBUF copies so the next qc's PV can start while the 1/denom broadcast takes
a zero-engine-cost DMA bounce through DRAM.
"""
import sys

sys.path.insert(0, "/opt/trn_rl_repo")

import numpy as np

import concourse.bacc as bacc
import concourse.mybir as mybir
import concourse.tile as tile
from concourse import bass_utils

F32 = mybir.dt.float32
BF16 = mybir.dt.bfloat16
F8 = mybir.dt.float8e4
U8 = mybir.dt.uint8
AF = mybir.ActivationFunctionType
ALU = mybir.AluOpType
AX = mybir.AxisListType
DR = mybir.MatmulPerfMode.DoubleRow

B, C, H, W = 4, 256, 64, 64
N = H * W                  # 4096 tokens
NQC = 8                    # query chunks of 512
QC = 512
NKT = 32                   # key tiles of 128
KT = 128
NPR = 16                   # key-tile pairs
INV_GN = 1.0 / 32
AM = 1.4426950408889634    # log2(e)
SAM = AM ** 0.5            # q,k pre-scaled so scores arrive as AM*s
BSH = 17.3123              # byte = max(AM*s, -BSH) + BSH
BIAS_ACT = (BSH - 56.0) / (8.0 * AM)
SCALE_ACT = 1.0 / (8.0 * AM)

_CACHE: dict = {}


def _build():
    nc = bacc.Bacc("TRN2", target_bir_lowering=False, debug=False,
                   enable_asserts=False)

    xb = nc.dram_tensor("xb", [4, 128, 2048], F8, kind="ExternalInput")
    wslb = nc.dram_tensor("wslb", [2, 128, 384], BF16, kind="ExternalInput")
    csts = nc.dram_tensor("csts", [128, 10], F32, kind="ExternalInput")
    selt = nc.dram_tensor("selt", [4, 128], F32, kind="ExternalInput")
    wptb = nc.dram_tensor("wptb", [128, 256], BF16, kind="ExternalInput")
    bqs = nc.dram_tensor("bqs", [1, 128], BF16, kind="ExternalInput")
    vtp8 = nc.dram_tensor("vtp8", [128, 4096], U8, kind="ExternalInput")
    yp = nc.dram_tensor("yp", [16, 128, 512], F32, kind="ExternalOutput")

    with tile.TileContext(nc) as tc:
        with (
            tc.tile_pool(name="cst", bufs=1) as cst,
            tc.tile_pool(name="big", bufs=1) as big,
            tc.tile_pool(name="pp", bufs=7) as pp,
            tc.tile_pool(name="sm", bufs=3) as sm,
            tc.tile_pool(name="dr", bufs=4, space="DRAM") as dr,
            tc.tile_pool(name="ps", bufs=3, space="PSUM") as ps,
            tc.tile_pool(name="po", bufs=2, space="PSUM") as po,
        ):
            pg = ps  # gen/proj psum tiles share the scores pool rotation
            # ---- constants ----
            W0B = cst.tile([128, 384], BF16, tag="w0")
            W1B = cst.tile([128, 384], BF16, tag="w1")
            WPB = cst.tile([128, 256], BF16, tag="wp")
            CST = cst.tile([128, 10], F32, tag="cst")
            BQK = CST[:, 0:2]
            GAM = CST[:, 2:4]
            BET = CST[:, 4:6]
            SEL = CST[:, 6:10]
            SELT = cst.tile([4, 128], F32, tag="selt")
            ONE = cst.tile([1, 128], F32, tag="one")
            ONEB = cst.tile([1, 128], BF16, tag="oneb")
            ONES512 = cst.tile([1, 512], BF16, tag="o512")
            EPS = cst.tile([128, 1], F32, tag="eps")
            BIA = cst.tile([128, 1], F32, tag="bia")
            WARM = cst.tile([1, 1], F32, tag="warm")
            nc.vector.memset(WARM[:], 1.0)
            nc.scalar.activation(WARM[:], WARM[:], AF.Sqrt)
            # PE warmup: keep the tensor engine busy from t=0 so the
            # p-state ramp reaches full clock before the real matmuls
            nc.vector.memset(ONES512[:], 1.0)
            wup = po.tile([128, QC], F32, tag="o", name="warmup")
            for _ in range(18):
                nc.tensor.matmul(wup[:], ONEB[0:1, 0:128], ONES512[0:1, :],
                                 start=True, stop=True)

            # fp8 attention operand tiles: partitions 0:32 head A (dim
            # 32*slot+p), 32:64 head B
            Q8 = cst.tile([64, 8, 2, 512], F8, tag="q8")
            K8 = cst.tile([64, 32, 2, 128], F8, tag="k8")
            VT8 = cst.tile([128, NPR, 2, 2, 128], F8, tag="vt8")
            BQS = cst.tile([1, 128], BF16, tag="bqs")

            # ---- load x (chunked, stats via one-pass bn_stats) ----
            NCH = 8
            CH = N // NCH   # 512
            X = [big.tile([128, N], F8, tag=f"x{cc}", name=f"X{cc}") for cc in range(2)]
            Hb = [big.tile([128, N], BF16, tag=f"hb{cc}", name=f"Hb{cc}") for cc in range(2)]
            BNS = [cst.tile([128, NCH * 6], F32, tag=f"bns{cc}", name=f"BNS{cc}") for cc in range(2)]
            MV = [cst.tile([128, 2], F32, tag=f"mv{cc}", name=f"MV{cc}") for cc in range(2)]
            ST = [cst.tile([128, 2], F32, tag=f"st{cc}", name=f"ST{cc}") for cc in range(2)]
            GS = cst.tile([4, 4], F32, tag="gs")
            gs_ps = pg.tile([4, 4], F32, tag="s")
            SX = cst.tile([128, 4], F32, tag="sx")
            SQ = cst.tile([128, 4], F32, tag="sq")
            for i in range(2):
                for cc in range(2):
                    dsl = slice(i * 2048, (i + 1) * 2048)
                    nc.sync.dma_start(X[cc][:, dsl], xb.ap()[cc * 2 + i])
                    for h in range(4):
                        j = 4 * i + h
                        sl = slice(j * CH, (j + 1) * CH)
                        if cc == 1 and j < 4:
                            # first-arriving half-1 slices on ACT (idle early);
                            # scratch into Hb[1] (overwritten later by real h)
                            nc.scalar.activation(
                                Hb[1][:, sl], X[1][:, sl], AF.Identity,
                                accum_out=SX[:, j:j + 1])
                            nc.scalar.activation(
                                Hb[1][:, sl], X[1][:, sl], AF.Square,
                                accum_out=SQ[:, j:j + 1])
                        else:
                            nc.vector.bn_stats(BNS[cc][:, 6 * j:6 * j + 6],
                                               X[cc][:, sl])
            # weights & consts (needed later than x)
            nc.sync.dma_start(BQS[:], bqs.ap())
            # VT ones column + zero padding via DMA'd host constant
            nc.sync.dma_start(VT8[:, :, :, :, 64:128].bitcast(U8), vtp8.ap())
            nc.vector.memset(EPS[:], 1e-5)
            nc.vector.memset(ONE[:], 1.0)
            nc.vector.memset(ONEB[:], 1.0)
            nc.vector.memset(BIA[:], BIAS_ACT)
            nc.sync.dma_start(CST[:], csts.ap())
            nc.sync.dma_start(SELT[:], selt.ap())
            nc.sync.dma_start(W0B[:], wslb.ap()[0])
            nc.sync.dma_start(W1B[:], wslb.ap()[1])
            nc.sync.dma_start(WPB[:], wptb.ap())
            for cc in range(2):
                if cc == 0:
                    nc.vector.bn_aggr(MV[0][:], BNS[0][:])
                    # ST = [mean_p, E[x^2]_p]
                    nc.vector.tensor_mul(ST[0][:, 1:2], MV[0][:, 0:1], MV[0][:, 0:1])
                    nc.vector.tensor_add(ST[0][:, 1:2], ST[0][:, 1:2], MV[0][:, 1:2])
                    nc.vector.tensor_copy(ST[0][:, 0:1], MV[0][:, 0:1])
                else:
                    # merge ACT sums (slices 0-3) with bn stats (slices 4-7)
                    nc.vector.bn_aggr(MV[1][:], BNS[1][:, 24:48])
                    sxs = cst.tile([128, 1], F32, tag="sxs")
                    sqs = cst.tile([128, 1], F32, tag="sqs")
                    nc.vector.reduce_sum(sxs[:], SX[:], axis=AX.X)
                    nc.vector.reduce_sum(sqs[:], SQ[:], axis=AX.X)
                    # mean_p = 0.5*mean_bn + sxs/4096
                    nc.vector.tensor_scalar_mul(ST[1][:, 0:1], MV[1][:, 0:1], 0.5)
                    nc.vector.tensor_scalar_mul(sxs[:], sxs[:], 1.0 / 4096.0)
                    nc.vector.tensor_add(ST[1][:, 0:1], ST[1][:, 0:1], sxs[:])
                    # E2_p = 0.25*(var_bn + mean_bn^2) + sqs/4096
                    nc.vector.tensor_mul(ST[1][:, 1:2], MV[1][:, 0:1], MV[1][:, 0:1])
                    nc.vector.tensor_add(ST[1][:, 1:2], ST[1][:, 1:2], MV[1][:, 1:2])
                    nc.vector.tensor_scalar_mul(ST[1][:, 1:2], ST[1][:, 1:2], 0.5)
                    nc.vector.tensor_scalar_mul(sqs[:], sqs[:], 1.0 / 4096.0)
                    nc.vector.tensor_add(ST[1][:, 1:2], ST[1][:, 1:2], sqs[:])
                nc.tensor.matmul(gs_ps[:, 2 * cc:2 * cc + 2], SEL,
                                 ST[cc][:], start=True, stop=True)
                nc.vector.tensor_copy(GS[:, 2 * cc:2 * cc + 2],
                                      gs_ps[:, 2 * cc:2 * cc + 2])

            # per-channel scale/shift: s = gamma/sqrt(var+eps), t = beta - mean*s
            gn_st = []
            for cc in range(2):
                pc_ps = pg.tile([128, 2], F32, tag="s", name=f"pc{cc}")
                nc.tensor.matmul(pc_ps[:], SELT[:], GS[:, 2 * cc:2 * cc + 2],
                                 start=True, stop=True)
                mean = cst.tile([128, 1], F32, tag=f"mean{cc}")
                var = cst.tile([128, 1], F32, tag=f"var{cc}")
                sd = cst.tile([128, 1], F32, tag=f"sd{cc}")
                s_t = cst.tile([128, 1], F32, tag=f"s{cc}")
                t_t = cst.tile([128, 1], F32, tag=f"t{cc}")
                nc.vector.tensor_scalar_mul(mean[:], pc_ps[:, 0:1], INV_GN)
                nc.vector.tensor_scalar_mul(var[:], pc_ps[:, 1:2], INV_GN)
                # var = E[x^2] - mean^2
                nc.vector.scalar_tensor_tensor(
                    out=sd[:], in0=mean[:], scalar=-1.0, in1=mean[:],
                    op0=ALU.mult, op1=ALU.mult)
                nc.vector.tensor_add(var[:], var[:], sd[:])
                nc.scalar.activation(sd[:], var[:], AF.Sqrt, bias=EPS[:])
                nc.vector.reciprocal(s_t[:], sd[:])
                nc.vector.tensor_mul(s_t[:], s_t[:], GAM[:, cc:cc + 1])
                nc.vector.scalar_tensor_tensor(
                    out=t_t[:], in0=mean[:], scalar=-1.0, in1=s_t[:],
                    op0=ALU.mult, op1=ALU.mult)
                nc.vector.tensor_add(t_t[:], t_t[:], BET[:, cc:cc + 1])
                gn_st.append((s_t, t_t))

            # h = x*s + t (bf16); split across Pool/ACT/DVE
            for i in range(4):
                sl = slice(i * 1024, (i + 1) * 1024)
                for cc in range(2):
                    s_t, t_t = gn_st[cc]
                    if i >= 1:
                        nc.gpsimd.tensor_scalar(
                            out=Hb[cc][:, sl], in0=X[cc][:, sl], scalar1=s_t[:],
                            scalar2=t_t[:], op0=ALU.mult, op1=ALU.add)
                    elif cc == 0:
                        nc.scalar.activation(Hb[0][:, sl], X[0][:, sl],
                                             AF.Identity, bias=t_t[:],
                                             scale=s_t[:])
                    else:
                        nc.vector.tensor_scalar(
                            out=Hb[1][:, sl], in0=X[1][:, sl], scalar1=s_t[:],
                            scalar2=t_t[:], op0=ALU.mult, op1=ALU.add)

            # ---- QKV emission: gen matmuls write the DoubleRow layout
            # directly ([64, slot, tok] psum); q-bias lands via a rank-1
            # matmul, the k-bias is softmax-invariant and dropped ----
            def emit_q_chunk(ch, part=2):
                # part 0: slot-0 matmuls, part 1: slot-1 + quantize,
                # part 2: everything (prologue use)
                tok = slice(ch * QC, (ch + 1) * QC)
                if part in (0, 2):
                    q_ps = pg.tile([64, 2, QC], F32, tag="s", name=f"q_ps{ch}")
                    _CACHE[f"qps{ch}"] = q_ps
                else:
                    q_ps = _CACHE.pop(f"qps{ch}")
                rng = {0: (0,), 1: (1,), 2: (0, 1)}[part]
                for s in rng:
                    cols = slice(64 * s, 64 * s + 64)
                    nc.tensor.matmul(q_ps[:, s, :], W0B[:, cols], Hb[0][:, tok],
                                     start=True, stop=False)
                    nc.tensor.matmul(q_ps[:, s, :], W1B[:, cols], Hb[1][:, tok],
                                     start=False, stop=False)
                    nc.tensor.matmul(q_ps[:, s, :], BQS[0:1, cols],
                                     ONES512[0:1, :], start=False, stop=True)
                if part in (1, 2):
                    nc.vector.tensor_copy(Q8[0:64, ch, :, :], q_ps[:])

            def emit_k_chunk(ch, eng):
                tok = slice(ch * QC, (ch + 1) * QC)
                k_ps = pg.tile([64, 2, 4, KT], F32, tag="s", name=f"k_ps{ch}")
                for s in range(2):
                    cols = slice(128 + 64 * s, 128 + 64 * s + 64)
                    nc.tensor.matmul(k_ps[:, s, :, :], W0B[:, cols],
                                     Hb[0][:, tok], start=True, stop=False)
                    nc.tensor.matmul(k_ps[:, s, :, :], W1B[:, cols],
                                     Hb[1][:, tok], start=False, stop=True)
                kin = k_ps[:].transpose([0, 2, 1, 3])
                if eng == "act":
                    nc.scalar.copy(K8[0:64, 4 * ch:4 * ch + 4, :, :], kin)
                else:
                    nc.vector.tensor_copy(K8[0:64, 4 * ch:4 * ch + 4, :, :], kin)

            def emit_vt_pair(pr):
                # vt4 cols (head, slot, dim): 8 matmuls of 64 cols; each
                # psum accumulation group closes before the next opens
                vt4 = pg.tile([128, 2, 2, 64], F32, tag="s", name=f"vt{pr}")
                for slv in range(2):
                    tok = slice((2 * pr + slv) * KT, (2 * pr + slv + 1) * KT)
                    for h in range(2):
                        vcols = slice(256 + 64 * h, 256 + 64 * h + 64)
                        for cc in range(2):
                            WB = W0B if cc == 0 else W1B
                            nc.tensor.matmul(vt4[:, h, slv, :], Hb[cc][:, tok],
                                             WB[:, vcols], start=(cc == 0),
                                             stop=(cc == 1))
                nc.scalar.copy(VT8[:, pr, :, :, 0:64], vt4[:])

            emit_q_chunk(0)
            emit_k_chunk(0, "act")
            emit_vt_pair(0)

            # ---- attention + projection ----
            # finish is split: finish_a (normalize chain start, frees O) runs
            # right at qc end; finish_b (attn mul + proj + y out) is delayed
            # into the next qc so the DRAM-bounce broadcast latency hides.
            pending_b = None
            for qc in range(NQC):
                O_A = po.tile([128, QC], F32, tag="o", name=f"O_A{qc}")
                O_B = po.tile([128, QC], F32, tag="o", name=f"O_B{qc}")
                p8 = None
                pv_queue = []
                for kt in range(NKT):
                    pr, slk = kt // 2, kt % 2
                    if qc == 0:
                        if kt % 4 == 0 and kt // 4 < 7:
                            emit_k_chunk(kt // 4 + 1, "act" if kt % 8 == 0 else "dve")
                        if slk == 0 and pr < NPR - 1:
                            emit_vt_pair(pr + 1)
                    if kt == 2 and pending_b is not None:
                        pending_b[0]()
                    if kt == 8 and pending_b is not None:
                        pending_b[1]()
                    if kt == 12 and pending_b is not None:
                        pending_b[2]()
                        pending_b = None
                    if kt == 30 and qc < NQC - 1:
                        emit_q_chunk(qc + 1)
                    s_ps = ps.tile([128, 1024], F32, tag="s", name=f"s{qc}_{kt}")
                    nc.tensor.matmul(s_ps[:, 0:512], K8[0:32, kt, :, :],
                                     Q8[0:32, qc, :, :], start=True, stop=True,
                                     perf_mode=DR)
                    nc.tensor.matmul(s_ps[:, 512:1024], K8[32:64, kt, :, :],
                                     Q8[32:64, qc, :, :], start=True, stop=True,
                                     perf_mode=DR)
                    if len(pv_queue) == 4:
                        pv_queue.pop(0)()
                    if slk == 0:
                        p8 = pp.tile([128, 2, 2, QC], F8, tag="p",
                                     name=f"p{qc}_{pr}")
                    # exp: split between ACT (true exp) and DVE (byte trick).
                    # DVE kts are never adjacent so the two engine streams
                    # overlap through the score pool rotation.
                    if kt % 2 == 1 and kt >= 3:
                        nc.vector.tensor_scalar(
                            out=p8[:].bitcast(U8)[:, :, slk, :], in0=s_ps[:],
                            scalar1=-BSH, scalar2=BSH, op0=ALU.max, op1=ALU.add)
                    else:
                        nc.scalar.activation(p8[:, :, slk, :], s_ps[:], AF.Exp,
                                             bias=BIA[:], scale=SCALE_ACT)

                    if slk == 1:
                        def _pv(pr=pr, p8=p8, O_A=O_A, O_B=O_B):
                            nc.tensor.matmul(O_A[:], VT8[:, pr, 0, :, :],
                                             p8[:, 0, :, :], start=(pr == 0),
                                             stop=(pr == NPR - 1), perf_mode=DR)
                            nc.tensor.matmul(O_B[:], VT8[:, pr, 1, :, :],
                                             p8[:, 1, :, :], start=(pr == 0),
                                             stop=(pr == NPR - 1), perf_mode=DR)
                        pv_queue.append(_pv)
                for f in pv_queue:
                    f()

                # finish_a: reciprocal of denominators + unnormalized O to
                # SBUF (frees the O psum fast so next qc's PV can start);
                # the broadcast goes through a DRAM bounce (no engine cost).
                rdt = BF16 if qc == NQC - 1 else F32
                rA = sm.tile([1, QC], rdt, tag="ra", name=f"rA{qc}")
                rB = sm.tile([1, QC], rdt, tag="rb", name=f"rB{qc}")
                o_sb = sm.tile([128, QC], F32, tag="osb", name=f"o_sb{qc}")

                def finish_a(qc=qc, O_A=O_A, O_B=O_B, rA=rA, rB=rB, o_sb=o_sb):
                    with nc.allow_low_precision(reason="1/denom bf16 tail"):
                        nc.vector.reciprocal(rA[:], O_A[64:65, :])
                        nc.vector.reciprocal(rB[:], O_B[64:65, :])
                    nc.scalar.copy(o_sb[0:64, :], O_A[0:64, :])
                    nc.vector.tensor_copy(o_sb[64:128, :], O_B[0:64, :])
                if qc == NQC - 1:
                    finish_a()
                    # tail: PE K=1 broadcast (shorter chain than the bounce);
                    # bf16 operands for full-rate matmul, product read from
                    # psum by the normalize directly
                    rAb = sm.tile([1, QC], BF16, tag="rab", name="rAb")
                    rBb = sm.tile([1, QC], BF16, tag="rbb", name="rBb")
                    nc.vector.tensor_copy(rAb[:], rA[:])
                    nc.vector.tensor_copy(rBb[:], rB[:])
                    bc_ps = ps.tile([128, 1024], F32, tag="s", name="bc_tail")
                    nc.tensor.matmul(bc_ps[0:64, 0:512], ONEB[0:1, 0:64],
                                     rAb[:], start=True, stop=True)
                    nc.tensor.matmul(bc_ps[64:128, 0:512], ONEB[0:1, 0:64],
                                     rBb[:], start=True, stop=True)
                    bcs = bc_ps[:, 0:512]
                else:
                    bcs_t = sm.tile([128, QC], F32, tag="bcs", name=f"bcs{qc}")

                    def finish_a(finish_a=finish_a, qc=qc, rA=rA, rB=rB,
                                 bcs_t=bcs_t):
                        finish_a()
                        rAd = dr.tile([1, QC], F32, tag="rad", name=f"rAd{qc}")
                        rBd = dr.tile([1, QC], F32, tag="rbd", name=f"rBd{qc}")
                        nc.sync.dma_start(rAd[:], rA[:])
                        nc.sync.dma_start(rBd[:], rB[:])
                        nc.sync.dma_start(bcs_t[0:64, :], rAd[:].broadcast_to((64, QC)))
                        nc.sync.dma_start(bcs_t[64:128, :], rBd[:].broadcast_to((64, QC)))
                    bcs = bcs_t[:]

                attn = sm.tile([128, QC], BF16, tag="attn", name=f"attn{qc}")

                def finish_b1(qc=qc, o_sb=o_sb, bcs=bcs, attn=attn):
                    nc.vector.tensor_mul(attn[:], o_sb[:], bcs)

                def finish_b2(qc=qc, attn=attn):
                    for half in range(2):
                        y_ps = ps.tile([128, QC], F32, tag="s",
                                       name=f"y_ps{qc}_{half}")
                        nc.tensor.matmul(y_ps[:], WPB[:, half * 128:(half + 1) * 128],
                                         attn[:], start=True, stop=True)
                        y_sb = sm.tile([128, QC], F32, tag="y",
                                       name=f"y_sb{qc}_{half}")
                        if qc == NQC - 1 and half == 1:
                            nc.vector.tensor_copy(y_sb[:], y_ps[:])
                        else:
                            nc.scalar.copy(y_sb[:], y_ps[:])
                        nc.sync.dma_start(yp.ap()[half * 8 + qc], y_sb[:])

                if qc == NQC - 1:
                    finish_b1()
                    finish_b2()
                    pending_b = None
                else:
                    pending_b = (finish_a, finish_b1, finish_b2)
            if pending_b is not None:
                pending_b[0]()
                pending_b[1]()
                pending_b[2]()

    nc.compile()
    return nc


def _get_nc():
    if "nc" not in _CACHE:
        _CACHE["nc"] = _build()
    return _CACHE["nc"]


def build_in_maps(x, gn_gamma, gn_beta, w_qkv, b_qkv, w_proj):
    import ml_dtypes
    sel_np = np.zeros((128, 4), np.float32)
    for c in range(128):
        sel_np[c, c // 32] = 1.0
    selt_np = sel_np.T.copy()
    gmt_np = np.stack([gn_gamma[0:128], gn_gamma[128:256]], axis=1)
    btt_np = np.stack([gn_beta[0:128], gn_beta[128:256]], axis=1)

    # VT8 cols 64:128 per (pr, h, sl): [1.0, 0 x 63]
    vtpat = np.zeros(64, ml_dtypes.float8_e4m3)
    vtpat[0] = 1.0
    vtp8_np = np.broadcast_to(vtpat.view(np.uint8), (128, 64, 64)).reshape(128, 4096)
    vtp8_np = np.ascontiguousarray(vtp8_np)
    # slot-head-interleaved row permutation for q/k: [A 0:32, B 0:32,
    # A 32:64, B 32:64] relative to this core's 128 channels
    qk_perm = np.concatenate([np.arange(0, 32), np.arange(64, 96),
                              np.arange(32, 64), np.arange(96, 128)])

    in_maps = []
    for core in range(8):
        b, j = core // 2, core % 2
        r0 = 128 * j
        wslb_np = np.zeros((2, 128, 384), np.float32)
        for cc in range(2):
            cols = slice(cc * 128, (cc + 1) * 128)
            wslb_np[cc, :, 0:128] = SAM * w_qkv[r0:r0 + 128, cols][qk_perm].T
            wslb_np[cc, :, 128:256] = SAM * w_qkv[256 + r0:256 + r0 + 128, cols][qk_perm].T
            wslb_np[cc, :, 256:320] = w_qkv[512 + r0:512 + r0 + 64, cols].T
            wslb_np[cc, :, 320:384] = w_qkv[512 + r0 + 64:512 + r0 + 128, cols].T
        bqs_np = (SAM * b_qkv[r0:r0 + 128][qk_perm]).reshape(1, 128)
        bqk_np = np.stack([b_qkv[r0:r0 + 128], b_qkv[256 + r0:256 + r0 + 128]],
                          axis=1)
        csts_np = np.concatenate([bqk_np, gmt_np, btt_np, sel_np], axis=1)
        xq = np.ascontiguousarray(
            x[b].reshape(2, 128, 2, 2048).transpose(0, 2, 1, 3)
            .reshape(4, 128, 2048).astype(ml_dtypes.float8_e4m3))
        in_maps.append({
            "xb": xq,
            "wslb": np.ascontiguousarray(wslb_np.astype(ml_dtypes.bfloat16)),
            "csts": np.ascontiguousarray(csts_np),
            "selt": selt_np,
            "wptb": np.ascontiguousarray(
                w_proj[:, r0:r0 + 128].T.astype(ml_dtypes.bfloat16)),
            "bqs": np.ascontiguousarray(bqs_np.astype(ml_dtypes.bfloat16)),
            "vtp8": vtp8_np,
        })

    return in_maps


def kernel(x, gn_gamma, gn_beta, w_qkv, b_qkv, w_proj, b_proj, **_unused):
    x = np.ascontiguousarray(np.asarray(x, dtype=np.float32))
    gn_gamma = np.asarray(gn_gamma, dtype=np.float32)
    gn_beta = np.asarray(gn_beta, dtype=np.float32)
    w_qkv = np.asarray(w_qkv, dtype=np.float32)
    b_qkv = np.asarray(b_qkv, dtype=np.float32)
    w_proj = np.asarray(w_proj, dtype=np.float32)
    b_proj = np.asarray(b_proj, dtype=np.float32)

    nc = _get_nc()
    in_maps = build_in_maps(x, gn_gamma, gn_beta, w_qkv, b_qkv, w_proj)
    res = bass_utils.run_bass_kernel_spmd(nc, in_maps, core_ids=list(range(8)))
    _CACHE["last_result"] = res

    # v-bias folds into the output bias: softmax weights sum to 1
    b_eff = b_proj + w_proj @ b_qkv[512:768]
    out = np.empty((B, C, N), np.float32)
    for b in range(B):
        ypsum = res.results[2 * b]["yp"] + res.results[2 * b + 1]["yp"]
        ypsum = ypsum.reshape(2, 8, 128, 512).transpose(0, 2, 1, 3).reshape(C, N)
        out[b] = ypsum + x[b].reshape(C, N) + b_eff[:, None]
    return out.reshape(B, C, H, W)
